# revision 1
# baseline (speedup 1.0000x reference)
"""Trainium2 Bass kernel for sigmoid-gated multi-head attention.

Reference computation (B=4, F=256, H=8, S=1024):
    qx  = q_input^T          (b, s, f)
    q   = qx @ Wq  -> (b, s, f, h)   [col fi*H + hi]
    k,v = kvx @ Wk / Wv
    attn = sigmoid(sqrt(F) * q.k)    per head
    wv   = attn @ v
    out  = relu(concat_heads(wv) @ Wz)   returned as (b, f, s)

Sharding: 8 cores = 4 batches x 2 query-sequence halves. Each core
computes the full pipeline (all 8 heads) for its (batch, s-half) slice,
including the final ReLU, so per-core outputs are disjoint slices of the
final output and no cross-core reduction is needed.  The cost is that
K/V projections are computed by both cores of a batch pair (~14% extra
matmul work vs. the ideal), in exchange for zero collectives.

All on-chip compute keeps the "transposed" layout (feature, sequence),
which matches the DRAM layout of q_input/kv_input and the required
output layout, so no transposes are ever needed:
    QT_h (f, i)  = Wq_h^T @ q_in       KT_h (f, j) = Wk_h^T @ kv_in
    V_h  (j, f)  = kv_in^T @ Wv_h
    attnT_h (j, i) = sigmoid(16 * KT_h^T_slice . QT_h)
    wvT_h (f, i) = V_h^T_slice @ attnT_h
    outT (fo, i) += Wz_h^T @ wvT_h     -> relu -> output slice

All matmuls run as fp32r (full PE rate at N>=256, ~1e-3 rel err).
Inputs are host-packed partition-major so every DRAM->SBUF transfer is
one large contiguous-per-partition DMA (stripes across all 16 SDMA
engines): one DMA for qin, one for kvin, one per head for all four
weight matrices, one for the output.
"""

import os
import sys

sys.path.insert(0, "/opt/trn_rl_repo")

import numpy as np

B, F, H, S = 4, 256, 8, 1024
HALF = S // 2  # query columns per core
QTR = HALF // 2
NCORES = 8
P = 128  # partitions

_cache = {}


def _build():
    import concourse.mybir as mybir
    import concourse.tile as tile
    from concourse import bacc

    dt = mybir.dt
    f32 = dt.float32
    # dtype for all matmul-input tensors (DRAM + SBUF).
    # fp16: halves DMA bytes + SBUF, full PE rate, ~10-bit mantissa.
    # fp32r: fp32 storage, PE rounds internally; walrus requires fp32r
    #   matmul operands to be *produced* as fp32r (copies carry the dtype).
    # fp32: exact but 4x slower on the PE.
    mm_mode = os.environ.get("ATTN_MM_DTYPE", "fp16")
    mdt = {"fp16": dt.float16, "fp32r": dt.float32r, "fp32": dt.float32}[mm_mode]
    AF = mybir.ActivationFunctionType

    nc = bacc.Bacc(None, target_bir_lowering=False)

    # all partition-major: [P, ...] with per-partition lines contiguous
    qin_d = nc.dram_tensor("qin", [P, 2, HALF], mdt, kind="ExternalInput")
    kvin_d = nc.dram_tensor("kvin", [P, 2, S], mdt, kind="ExternalInput")
    # per head: [wq|wk|wv|wz][f_in chunk][f_out]
    w_d = nc.dram_tensor("w", [H, P, 4, 2, F], mdt, kind="ExternalInput")
    odt = dt.float16 if mm_mode == "fp16" else f32
    out_d = nc.dram_tensor("out", [P, 2, HALF], odt, kind="ExternalOutput")

    with tile.TileContext(nc) as tc:
        with (
            tc.tile_pool(name="io", bufs=1) as io_pool,
            tc.tile_pool(name="wts", bufs=2) as w_pool,
            tc.tile_pool(name="qkv", bufs=2) as qkv_pool,
            tc.tile_pool(name="attn", bufs=2) as attn_pool,
            tc.tile_pool(name="ps", bufs=6, space="PSUM") as ps_pool,
            tc.tile_pool(name="ops", bufs=1, space="PSUM") as out_ps_pool,
        ):
            # PE pre-warm: dummy matmuls on a zeroed bf16 tile keep the PE
            # busy through its HAM activity window while the first input
            # DMAs are in flight, so the real matmuls start at 2.4 GHz
            # instead of paying the ~3.4us half-clock ramp. Two PSUM slots
            # alternate so consecutive dummies pipeline instead of
            # serializing on a same-bank WAW dependency.
            nwarm = int(os.environ.get("ATTN_NWARM", "16"))
            if nwarm:
                warm = io_pool.tile([P, HALF], dt.bfloat16 if mm_mode != "fp32" else f32, tag="warm")
                nc.vector.memset(warm[:], 0.0)
                wps = [ps_pool.tile([P, HALF], f32, tag="ps", name=f"wps{i}") for i in range(2)]
                for i in range(nwarm):
                    nc.tensor.matmul(
                        wps[i % 2][:], warm[:, :P], warm[:], start=True, stop=True
                    )

            qin = io_pool.tile([P, 2, HALF], mdt, tag="qin")
            # kvin split by f_in chunk into two tiles so K/V matmuls on
            # chunk 0 can start as soon as the first half arrives.
            kvin = [io_pool.tile([P, S], mdt, tag=f"kvin{c}", name=f"kvin{c}") for c in range(2)]
            # qin on the SP HWDGE ring, kvin on the ACT ring: both input
            # transfers start immediately and run in parallel.
            nc.sync.dma_start(qin[:], qin_d[:])
            nc.scalar.dma_start(kvin[0][:], kvin_d[:, 0])
            nc.scalar.dma_start(kvin[1][:], kvin_d[:, 1])

            # persistent accumulator for the output projection: 2 banks
            out_ps = out_ps_pool.tile([P, 2, HALF], f32, tag="out_ps")

            def q_proj(h):
                """Emit Q-projection matmuls for head h; returns qt tile."""
                wq = w_pool.tile([P, 2, F], mdt, tag="wq", name=f"wq{h}")
                nc.sync.dma_start(wq[:], w_d[h, :, 0])
                qt = qkv_pool.tile([P, 2, HALF], mdt, tag="qt", name=f"qt{h}")
                for t in range(2):
                    ps = ps_pool.tile([P, HALF], f32, tag="ps", name=f"psq{h}{t}")
                    for c in range(2):
                        nc.tensor.matmul(
                            ps[:],
                            wq[:, c, P * t : P * (t + 1)],
                            qin[:, c, :],
                            start=(c == 0),
                            stop=(c == 1),
                        )
                    if t == 0:
                        nc.vector.tensor_copy(qt[:, t, :], ps[:])
                    else:
                        nc.scalar.activation(qt[:, t, :], ps[:], AF.Copy)
                return qt

            qt_next = q_proj(0)
            for h in range(H):
                qt = qt_next
                wk = w_pool.tile([P, 2, F], mdt, tag="wk")
                nc.scalar.dma_start(wk[:], w_d[h, :, 1])
                wvz = w_pool.tile([P, 2, 2, F], mdt, tag="wvz")
                nc.scalar.dma_start(wvz[:], w_d[h, :, 2:4])
                wv = wvz[:, 0]
                wz = wvz[:, 1]

                # KT_h (f 2x128, j 1024) = Wk_h^T @ kvin
                kt = qkv_pool.tile([P, 2, S], mdt, tag="kt")
                for t in range(2):
                    for n in range(2):
                        ps = ps_pool.tile([P, HALF], f32, tag="ps")
                        for c in range(2):
                            nc.tensor.matmul(
                                ps[:],
                                wk[:, c, P * t : P * (t + 1)],
                                kvin[c][:, HALF * n : HALF * (n + 1)],
                                start=(c == 0),
                                stop=(c == 1),
                            )
                        if (t + n) % 2 == 0:
                            nc.vector.tensor_copy(
                                kt[:, t, HALF * n : HALF * (n + 1)], ps[:]
                            )
                        else:
                            nc.scalar.activation(
                                kt[:, t, HALF * n : HALF * (n + 1)], ps[:], AF.Copy
                            )

                # V_h (j 8x128, f 256) = kvin^T @ Wv_h
                v = qkv_pool.tile([P, H, F], mdt, tag="v")
                for jb in range(8):
                    ps = ps_pool.tile([P, HALF], f32, tag="ps")
                    for c in range(2):
                        nc.tensor.matmul(
                            ps[:, :F],
                            kvin[c][:, P * jb : P * (jb + 1)],
                            wv[:, c, :],
                            start=(c == 0),
                            stop=(c == 1),
                        )
                    if jb % 2 == 0:
                        nc.vector.tensor_copy(v[:, jb, :], ps[:, :F])
                    else:
                        nc.scalar.activation(v[:, jb, :], ps[:, :F], AF.Copy)

                # attnT_h (j 8x128, i 512) = sigmoid(16 * KT_slice^T @ QT)
                atn = attn_pool.tile([P, 8, HALF], mdt, tag="atn")
                for jb in range(8):
                    ps = ps_pool.tile([P, HALF], f32, tag="ps")
                    for c in range(2):
                        nc.tensor.matmul(
                            ps[:],
                            kt[:, c, P * jb : P * (jb + 1)],
                            qt[:, c, :],
                            start=(c == 0),
                            stop=(c == 1),
                        )
                    nc.scalar.activation(atn[:, jb, :], ps[:], AF.Sigmoid, scale=16.0)

                # software-pipeline: emit next head's Q projection here so
                # the PE has ready work across the head boundary.
                if h + 1 < H:
                    qt_next = q_proj(h + 1)

                # wvT_h (f 2x128, i 512) = V_slice^T @ attnT, with the
                # output-projection matmuls for each f-chunk interleaved
                # right after that chunk's PSUM->SBUF copy so the PE never
                # waits on the copy latency.
                wvt = qkv_pool.tile([P, 2, HALF], mdt, tag="wvt")
                for c in range(2):
                    if h == H - 1 and c == 1:
                        # very last chain: split by i-half so the copy and
                        # final projection of half 0 overlap half 1's
                        # accumulation, shortening the kernel tail.
                        for ih in range(2):
                            ps = ps_pool.tile(
                                [P, HALF], f32, tag="ps", name=f"pswv{ih}"
                            )
                            sl = slice(F * ih, F * (ih + 1))
                            for jb in range(8):
                                nc.tensor.matmul(
                                    ps[:, :F],
                                    v[:, jb, P * c : P * (c + 1)],
                                    atn[:, jb, sl],
                                    start=(jb == 0),
                                    stop=(jb == 7),
                                )
                            nc.vector.tensor_copy(wvt[:, c, sl], ps[:, :F])
                            for t in range(2):
                                nc.tensor.matmul(
                                    out_ps[:, t, sl],
                                    wz[:, c, P * t : P * (t + 1)],
                                    wvt[:, c, sl],
                                    start=False,
                                    stop=(ih == 1),
                                )
                        continue
                    ps = ps_pool.tile([P, HALF], f32, tag="ps")
                    for jb in range(8):
                        nc.tensor.matmul(
                            ps[:],
                            v[:, jb, P * c : P * (c + 1)],
                            atn[:, jb, :],
                            start=(jb == 0),
                            stop=(jb == 7),
                        )
                    nc.vector.tensor_copy(wvt[:, c, :], ps[:])
                    # outT (fo 2x128, i 512) += Wz_h[c-chunk]^T @ wvT[c]
                    for t in range(2):
                        nc.tensor.matmul(
                            out_ps[:, t, :],
                            wz[:, c, P * t : P * (t + 1)],
                            wvt[:, c, :],
                            start=(h == 0 and c == 0),
                            stop=False,
                        )

            # tail: the two fo-halves finish at different times; run their
            # ReLUs on different engines in parallel and overlap the first
            # output DMA with the second ReLU.
            out_sb = io_pool.tile([P, 2, HALF], odt, tag="out_sb")
            nc.vector.tensor_relu(out_sb[:, 0, :], out_ps[:, 0, :])
            nc.sync.dma_start(out_d[:, 0], out_sb[:, 0, :])
            nc.scalar.activation(out_sb[:, 1, :], out_ps[:, 1, :], AF.Relu)
            nc.sync.dma_start(out_d[:, 1], out_sb[:, 1, :])

    nc.compile()
    return nc


def _get_nc():
    key = os.environ.get("ATTN_MM_DTYPE", "fp16")
    if key not in _cache:
        _cache[key] = _build()
    return _cache[key]


def _make_in_maps(inputs):
    ndt = (
        np.float16
        if os.environ.get("ATTN_MM_DTYPE", "fp16") == "fp16"
        else np.float32
    )
    q_input = np.asarray(inputs["q_input"], dtype=np.float32)
    kv_input = np.asarray(inputs["kv_input"], dtype=np.float32)

    # Wq/Wk/Wv [f_in, fo*H] (col fi*H+hi) -> [h, f_in(chunk c, p), fo]
    def cols_by_head(W):
        return np.asarray(W, dtype=np.float32).reshape(2, P, F, H).transpose(3, 0, 1, 2)

    # Wz [f*H, fo] (row fi*H+hi) -> [h, f(chunk c, p), fo]
    WzR = (
        np.asarray(inputs["Wz"], dtype=np.float32)
        .reshape(2, P, H, F)
        .transpose(2, 0, 1, 3)
    )
    # stack to [H, 4, 2, P, F] then to partition-major [H, P, 4, 2, F]
    WALL = np.stack(
        [
            cols_by_head(inputs["Wq"]),
            cols_by_head(inputs["Wk"]),
            cols_by_head(inputs["Wv"]),
            WzR,
        ],
        axis=1,
    )  # [H, 4, 2, P, F]
    WALL = np.ascontiguousarray(WALL.transpose(0, 3, 1, 2, 4), dtype=ndt)  # [H, P, 4, 2, F]

    in_maps = []
    for c in range(NCORES):
        b, half = divmod(c, 2)
        # q_input[b] (256, 1024) -> [p, chunk, i-half]
        qb = q_input[b].reshape(2, P, S)
        qin = np.ascontiguousarray(
            qb[:, :, half * HALF : (half + 1) * HALF].transpose(1, 0, 2), dtype=ndt
        )
        kvin = np.ascontiguousarray(
            kv_input[b].reshape(2, P, S).transpose(1, 0, 2), dtype=ndt
        )
        in_maps.append({"qin": qin, "kvin": kvin, "w": WALL})
    return in_maps


def kernel(q_input, kv_input, Wq, Wk, Wv, Wz, **kw):
    from concourse.bass_utils import run_bass_kernel_spmd

    nc = _get_nc()
    in_maps = _make_in_maps(
        {
            "q_input": q_input,
            "kv_input": kv_input,
            "Wq": Wq,
            "Wk": Wk,
            "Wv": Wv,
            "Wz": Wz,
        }
    )

    res = run_bass_kernel_spmd(nc, in_maps, core_ids=list(range(NCORES)))

    out = np.empty((B, F, S), dtype=np.float32)
    for c in range(NCORES):
        b, half = divmod(c, 2)
        # out dram [p, chunk, i] -> out[b, chunk*128+p, half*512+i]
        o = np.asarray(res.results[c]["out"], dtype=np.float32)  # (P, 2, HALF)
        out[b, :, half * HALF : (half + 1) * HALF] = o.transpose(1, 0, 2).reshape(
            F, HALF
        )
    return out



# revision 2
# speedup vs baseline: 1.0325x; 1.0325x over previous
"""Trainium2 Bass kernel for sigmoid-gated multi-head attention.

Reference computation (B=4, F=256, H=8, S=1024):
    qx  = q_input^T          (b, s, f)
    q   = qx @ Wq  -> (b, s, f, h)   [col fi*H + hi]
    k,v = kvx @ Wk / Wv
    attn = sigmoid(sqrt(F) * q.k)    per head
    wv   = attn @ v
    out  = relu(concat_heads(wv) @ Wz)   returned as (b, f, s)

Weight-folding: because attention scores and the output are bilinear in
the projections, the K and V projections can be folded into per-head
256x256 matrices computed on the host for free:
    A_h = Wq_h @ Wk_h^T          qkt_h = qin^T A_h kvin = (A_h^T qin)^T kvin
    B_h = Wv_h @ Wz_h            out  = relu(sum_h B_h^T (kvin @ attnT_h))
This removes the K and V projection matmuls entirely (-28% PE work) and
with them the duplicated K/V compute across the query-half core pair.

Sharding: 8 cores = 4 batches x 2 query-sequence halves. Each core
computes all 8 heads for its (batch, s-half) slice including the final
ReLU, so per-core outputs are disjoint slices of the final output and
no cross-core communication is needed.

Per head (all matmuls fp16, N=512, warm ~216ns):
    qt'_h (f, i)   = A_h^T @ qin                  4 MMs
    attnT_h (j, i) = sigmoid(16 * kvin_sl^T qt')  16 MMs
    u_h   (f, i)   = kvinT_sl^T @ attnT_h         16 MMs (acc over j)
    outT (fo, i)  += B_h^T @ u_h                   4 MMs (acc over h)
Inputs are host-packed partition-major so every DRAM->SBUF transfer is
one large contiguous-per-partition DMA; kvin is supplied in both
feature-major and sequence-major layouts to avoid on-chip transposes.
"""

import os
import sys

sys.path.insert(0, "/opt/trn_rl_repo")

import numpy as np

B, F, H, S = 4, 256, 8, 1024
HALF = S // 2  # query columns per core
NCORES = 8
P = 128  # partitions

_cache = {}


def _build():
    import concourse.mybir as mybir
    import concourse.tile as tile
    from concourse import bacc

    dt = mybir.dt
    f32 = dt.float32
    mm_mode = os.environ.get("ATTN_MM_DTYPE", "fp16")
    mdt = {"fp16": dt.float16, "fp32r": dt.float32r, "fp32": dt.float32}[mm_mode]
    AF = mybir.ActivationFunctionType

    nc = bacc.Bacc(None, target_bir_lowering=False)

    # all partition-major: [P, ...] with per-partition lines contiguous
    qin_d = nc.dram_tensor("qin", [P, 2, HALF], mdt, kind="ExternalInput")
    kvin_d = nc.dram_tensor("kvin", [P, 2, S], mdt, kind="ExternalInput")
    kvt_d = nc.dram_tensor("kvt", [P, 8, F], mdt, kind="ExternalInput")
    # per head: [A|B][f_in chunk][f_out]
    w_d = nc.dram_tensor("w", [H, P, 2, 2, F], mdt, kind="ExternalInput")
    odt = dt.float16 if mm_mode == "fp16" else f32
    out_d = nc.dram_tensor("out", [P, 2, HALF], odt, kind="ExternalOutput")

    with tile.TileContext(nc) as tc:
        with (
            tc.tile_pool(name="io", bufs=1) as io_pool,
            tc.tile_pool(name="wts", bufs=2) as w_pool,
            tc.tile_pool(name="qkv", bufs=2) as qkv_pool,
            tc.tile_pool(name="attn", bufs=2) as attn_pool,
            tc.tile_pool(name="ps", bufs=6, space="PSUM") as ps_pool,
            tc.tile_pool(name="ops", bufs=1, space="PSUM") as out_ps_pool,
        ):
            qin = io_pool.tile([P, 2, HALF], mdt, tag="qin")
            kvin = io_pool.tile([P, 2, S], mdt, tag="kvin")
            kvt = io_pool.tile([P, 8, F], mdt, tag="kvt")
            # qin+first weights on the SP HWDGE ring, kvin/kvt on the ACT
            # ring: all input transfers start immediately and in parallel.
            nc.sync.dma_start(qin[:], qin_d[:])
            nc.scalar.dma_start(kvin[:], kvin_d[:])
            nc.scalar.dma_start(kvt[:], kvt_d[:])

            # PE pre-warm: dummy matmuls on a zeroed bf16 tile keep the PE
            # busy through its HAM activity window while the first input
            # DMAs are in flight, so the real matmuls start at 2.4 GHz
            # instead of paying the ~3.4us half-clock ramp.
            nwarm = int(os.environ.get("ATTN_NWARM", "16"))
            if nwarm:
                warm = io_pool.tile(
                    [P, HALF], dt.bfloat16 if mm_mode != "fp32" else f32, tag="warm"
                )
                nc.vector.memset(warm[:], 0.0)
                wps = [
                    ps_pool.tile([P, HALF], f32, tag="ps", name=f"wps{i}")
                    for i in range(2)
                ]
                for i in range(nwarm):
                    nc.tensor.matmul(
                        wps[i % 2][:], warm[:, :P], warm[:], start=True, stop=True
                    )

            # persistent accumulator for the output projection: 2 banks
            out_ps = out_ps_pool.tile([P, 2, HALF], f32, tag="out_ps")

            def load_w(h):
                w = w_pool.tile([P, 2, 2, F], mdt, tag="w", name=f"w{h}")
                (nc.sync if h % 2 == 0 else nc.scalar).dma_start(w[:], w_d[h])
                return w

            def q_proj(h, w):
                """qt'_h = A_h^T @ qin; A = w[:, 0]."""
                qt = qkv_pool.tile([P, 2, HALF], mdt, tag="qt", name=f"qt{h}")
                for t in range(2):
                    ps = ps_pool.tile([P, HALF], f32, tag="ps", name=f"psq{h}{t}")
                    for c in range(2):
                        nc.tensor.matmul(
                            ps[:],
                            w[:, 0, c, P * t : P * (t + 1)],
                            qin[:, c, :],
                            start=(c == 0),
                            stop=(c == 1),
                        )
                    if t == 0:
                        nc.vector.tensor_copy(qt[:, t, :], ps[:])
                    else:
                        nc.scalar.activation(qt[:, t, :], ps[:], AF.Copy)
                return qt

            w_next = load_w(0)
            qt_next = q_proj(0, w_next)
            for h in range(H):
                w = w_next
                qt = qt_next
                # attnT_h (j 8x128, i 512) = sigmoid(16 * kvin_sl^T @ qt')
                atn = attn_pool.tile([P, 8, HALF], mdt, tag="atn")
                for jb in range(8):
                    ps = ps_pool.tile([P, HALF], f32, tag="ps")
                    for c in range(2):
                        nc.tensor.matmul(
                            ps[:],
                            kvin[:, c, P * jb : P * (jb + 1)],
                            qt[:, c, :],
                            start=(c == 0),
                            stop=(c == 1),
                        )
                    nc.scalar.activation(atn[:, jb, :], ps[:], AF.Sigmoid, scale=16.0)

                # software-pipeline: next head's weight DMA + Q projection
                # here so the PE has ready work across the head boundary.
                if h + 1 < H:
                    w_next = load_w(h + 1)
                    qt_next = q_proj(h + 1, w_next)

                # u_h (f 2x128, i 512) = kvinT_sl^T @ attnT (acc over j),
                # then outT (fo 2x128, i 512) += B_h^T @ u_h right after
                # each chunk's PSUM->SBUF copy.
                u = qkv_pool.tile([P, 2, HALF], mdt, tag="u")
                for t in range(2):
                    ps = ps_pool.tile([P, HALF], f32, tag="ps", name=f"psu{h}{t}")
                    for jb in range(8):
                        nc.tensor.matmul(
                            ps[:],
                            kvt[:, jb, P * t : P * (t + 1)],
                            atn[:, jb, :],
                            start=(jb == 0),
                            stop=(jb == 7),
                        )
                    if t == 0:
                        nc.vector.tensor_copy(u[:, t, :], ps[:])
                    else:
                        nc.scalar.activation(u[:, t, :], ps[:], AF.Copy)
                # outT += B_h^T @ u ; B = w[:, 1]
                for t in range(2):
                    for c in range(2):
                        nc.tensor.matmul(
                            out_ps[:, t, :],
                            w[:, 1, c, P * t : P * (t + 1)],
                            u[:, c, :],
                            start=(h == 0 and c == 0),
                            stop=(h == H - 1 and c == 1),
                        )

            # tail: run the two fo-halves' ReLUs on different engines in
            # parallel and overlap the first output DMA with the second.
            out_sb = io_pool.tile([P, 2, HALF], odt, tag="out_sb")
            nc.vector.tensor_relu(out_sb[:, 0, :], out_ps[:, 0, :])
            nc.sync.dma_start(out_d[:, 0], out_sb[:, 0, :])
            nc.scalar.activation(out_sb[:, 1, :], out_ps[:, 1, :], AF.Relu)
            nc.sync.dma_start(out_d[:, 1], out_sb[:, 1, :])

    nc.compile()
    return nc


def _get_nc():
    key = os.environ.get("ATTN_MM_DTYPE", "fp16")
    if key not in _cache:
        _cache[key] = _build()
    return _cache[key]


def _make_in_maps(inputs):
    ndt = (
        np.float16
        if os.environ.get("ATTN_MM_DTYPE", "fp16") == "fp16"
        else np.float32
    )
    q_input = np.asarray(inputs["q_input"], dtype=np.float32)
    kv_input = np.asarray(inputs["kv_input"], dtype=np.float32)

    # Wq/Wk/Wv [f_in, fo*H] (col fi*H + hi) -> [f_in, fo, h]
    WqH = np.asarray(inputs["Wq"], dtype=np.float32).reshape(F, F, H)
    WkH = np.asarray(inputs["Wk"], dtype=np.float32).reshape(F, F, H)
    WvH = np.asarray(inputs["Wv"], dtype=np.float32).reshape(F, F, H)
    # Wz [f*H, fo] (row fi*H + hi) -> [f_in, h, fo]
    WzH = np.asarray(inputs["Wz"], dtype=np.float32).reshape(F, H, F)

    # fold: A_h = Wq_h Wk_h^T, B_h = Wv_h Wz_h  (both [f_in=256, f_out=256])
    # pack each as [P, chunk, f_out]; stack to [H, P, 2(A|B), 2(chunk), F]
    WALL = np.empty((H, P, 2, 2, F), dtype=ndt)
    for h in range(H):
        A = WqH[:, :, h] @ WkH[:, :, h].T
        Bm = WvH[:, :, h] @ WzH[:, h, :]
        WALL[h, :, 0] = A.reshape(2, P, F).transpose(1, 0, 2)
        WALL[h, :, 1] = Bm.reshape(2, P, F).transpose(1, 0, 2)

    in_maps = []
    kvt_c = {}
    kvin_c = {}
    for c in range(NCORES):
        b, half = divmod(c, 2)
        # q_input[b] (256, 1024) -> [p, chunk, i-half]
        qb = q_input[b].reshape(2, P, S)
        qin = np.ascontiguousarray(
            qb[:, :, half * HALF : (half + 1) * HALF].transpose(1, 0, 2), dtype=ndt
        )
        if b not in kvin_c:
            kvin_c[b] = np.ascontiguousarray(
                kv_input[b].reshape(2, P, S).transpose(1, 0, 2), dtype=ndt
            )
            # kvin^T (1024, 256) -> [p, j-block, f]
            kvt_c[b] = np.ascontiguousarray(
                kv_input[b].T.reshape(8, P, F).transpose(1, 0, 2), dtype=ndt
            )
        in_maps.append(
            {"qin": qin, "kvin": kvin_c[b], "kvt": kvt_c[b], "w": WALL}
        )
    return in_maps


def kernel(q_input, kv_input, Wq, Wk, Wv, Wz, **kw):
    from concourse.bass_utils import run_bass_kernel_spmd

    nc = _get_nc()
    in_maps = _make_in_maps(
        {
            "q_input": q_input,
            "kv_input": kv_input,
            "Wq": Wq,
            "Wk": Wk,
            "Wv": Wv,
            "Wz": Wz,
        }
    )

    res = run_bass_kernel_spmd(nc, in_maps, core_ids=list(range(NCORES)))

    out = np.empty((B, F, S), dtype=np.float32)
    for c in range(NCORES):
        b, half = divmod(c, 2)
        # out dram [p, chunk, i] -> out[b, chunk*128+p, half*512+i]
        o = np.asarray(res.results[c]["out"], dtype=np.float32)  # (P, 2, HALF)
        out[b, :, half * HALF : (half + 1) * HALF] = o.transpose(1, 0, 2).reshape(
            F, HALF
        )
    return out


# revision 6
# speedup vs baseline: 1.2987x; 1.2579x over previous
"""Trainium2 Bass kernel for sigmoid-gated multi-head attention.

Reference computation (B=4, F=256, H=8, S=1024):
    qx  = q_input^T          (b, s, f)
    q   = qx @ Wq  -> (b, s, f, h)   [col fi*H + hi]
    k,v = kvx @ Wk / Wv
    attn = sigmoid(sqrt(F) * q.k)    per head
    wv   = attn @ v
    out  = relu(concat_heads(wv) @ Wz)   returned as (b, f, s)

Weight-folding: because attention scores and the output are bilinear in
the projections, the K and V projections can be folded into per-head
256x256 matrices computed on the host for free:
    A_h = Wq_h @ Wk_h^T          qkt_h = qin^T A_h kvin = (A_h^T qin)^T kvin
    B_h = Wv_h @ Wz_h            out  = relu(sum_h B_h^T (kvin @ attnT_h))
This removes the K and V projection matmuls entirely (-28% PE work) and
with them the duplicated K/V compute across the query-half core pair.

Sharding: 8 cores = 4 batches x 2 query-sequence halves. Each core
computes all 8 heads for its (batch, s-half) slice including the final
ReLU, so per-core outputs are disjoint slices of the final output and
no cross-core communication is needed.

Per head (all matmuls fp16, N=512, warm ~216ns):
    qt'_h (f, i)   = A_h^T @ qin                  4 MMs
    attnT_h (j, i) = sigmoid(16 * kvin_sl^T qt')  16 MMs
    u_h   (f, i)   = kvinT_sl^T @ attnT_h         16 MMs (acc over j)
    outT (fo, i)  += B_h^T @ u_h                   4 MMs (acc over h)
Inputs are host-packed partition-major so every DRAM->SBUF transfer is
one large contiguous-per-partition DMA; kvin is supplied in both
feature-major and sequence-major layouts to avoid on-chip transposes.
"""

import os
import sys

sys.path.insert(0, "/opt/trn_rl_repo")

import numpy as np

B, F, H, S = 4, 256, 8, 1024
HALF = S // 2  # query columns per core
NCORES = 8
P = 128  # partitions

_cache = {}


def _build():
    import concourse.mybir as mybir
    import concourse.tile as tile
    from concourse import bacc

    dt = mybir.dt
    f32 = dt.float32
    mm_mode = os.environ.get("ATTN_MM_DTYPE", "fp16")
    mdt = {"fp16": dt.float16, "fp32r": dt.float32r, "fp32": dt.float32}[mm_mode]
    AF = mybir.ActivationFunctionType

    nc = bacc.Bacc(None, target_bir_lowering=False)

    # all partition-major: [P, ...] with per-partition lines contiguous
    qin_d = nc.dram_tensor("qin", [P, 2, HALF], mdt, kind="ExternalInput")
    kvin_d = nc.dram_tensor("kvin", [P, 2, S], mdt, kind="ExternalInput")
    kvt_d = nc.dram_tensor("kvt", [P, 8, F], mdt, kind="ExternalInput")
    # per head: [A|B][f_in chunk][f_out]
    w_d = nc.dram_tensor("w", [H, P, 2, 2, F], mdt, kind="ExternalInput")
    odt = dt.float16 if mm_mode == "fp16" else f32
    out_d = nc.dram_tensor("out", [P, 2, HALF], odt, kind="ExternalOutput")

    with tile.TileContext(nc) as tc:
        with (
            tc.tile_pool(name="io", bufs=1) as io_pool,
            tc.tile_pool(name="wts", bufs=3) as w_pool,
            tc.tile_pool(name="qkv", bufs=2) as qkv_pool,
            tc.tile_pool(name="attn", bufs=2) as attn_pool,
            tc.tile_pool(name="ps", bufs=6, space="PSUM") as ps_pool,
            tc.tile_pool(name="ops", bufs=1, space="PSUM") as out_ps_pool,
        ):
            qin = io_pool.tile([P, 2, HALF], mdt, tag="qin")
            kvin = io_pool.tile([P, 2, S], mdt, tag="kvin")
            kvt = io_pool.tile([P, 8, F], mdt, tag="kvt")
            # qin+first weights on the SP HWDGE ring, kvin/kvt on the ACT
            # ring: all input transfers start immediately and in parallel.
            nc.sync.dma_start(qin[:], qin_d[:])
            nc.scalar.dma_start(kvin[:], kvin_d[:])
            nc.scalar.dma_start(kvt[:], kvt_d[:])

            # PE pre-warm: dummy matmuls on a zeroed bf16 tile keep the PE
            # busy through its HAM activity window while the first input
            # DMAs are in flight, so the real matmuls start at 2.4 GHz
            # instead of paying the ~3.4us half-clock ramp.
            nwarm = int(os.environ.get("ATTN_NWARM", "12"))
            if nwarm:
                warm = io_pool.tile(
                    [P, HALF], dt.bfloat16 if mm_mode != "fp32" else f32, tag="warm"
                )
                nc.gpsimd.memset(warm[:], 0.0)
                wps = [
                    ps_pool.tile([P, HALF], f32, tag="ps", name=f"wps{i}")
                    for i in range(2)
                ]
                for i in range(nwarm):
                    nc.tensor.matmul(
                        wps[i % 2][:], warm[:, :P], warm[:], start=True, stop=True
                    )

            # persistent accumulator for the output projection: 2 banks
            out_ps = out_ps_pool.tile([P, 2, HALF], f32, tag="out_ps")

            def load_w(h):
                w = w_pool.tile([P, 2, 2, F], mdt, tag="w", name=f"w{h}")
                (nc.sync if h % 2 == 0 else nc.scalar).dma_start(w[:], w_d[h])
                return w

            def q_proj(h, w):
                """qt'_h = A_h^T @ qin; A = w[:, 0]."""
                qt = qkv_pool.tile([P, 2, HALF], mdt, tag="qt", name=f"qt{h}")
                for t in range(2):
                    ps = ps_pool.tile([P, HALF], f32, tag="ps", name=f"psq{h}{t}")
                    for c in range(2):
                        nc.tensor.matmul(
                            ps[:],
                            w[:, 0, c, P * t : P * (t + 1)],
                            qin[:, c, :],
                            start=(c == 0),
                            stop=(c == 1),
                        )
                    nc.vector.tensor_copy(qt[:, t, :], ps[:])
                return qt

            def out_proj(h, w, u):
                """outT += B_h^T @ u ; B = w[:, 1].  c-major so the pair
                needing only u[:,0] runs while u[:,1]'s copy lands."""
                for c in range(2):
                    for t in range(2):
                        nc.tensor.matmul(
                            out_ps[:, t, :],
                            w[:, 1, c, P * t : P * (t + 1)],
                            u[:, c, :],
                            start=(h == 0 and c == 0),
                            stop=(h == H - 1 and c == 1),
                        )

            # software pipeline: weights prefetched a full head ahead; the
            # out-projection of head h-1 is emitted after head h's attention
            # matmuls so its PSUM->SBUF u-copies are long done when the PE
            # reaches it.
            w_next = load_w(0)
            qt_next = q_proj(0, w_next)
            out_pending = None
            for h in range(H):
                w = w_next
                qt = qt_next
                if h + 1 < H:
                    w_next = load_w(h + 1)
                # attnT_h (j 8x128, i 512) = sigmoid(16 * kvin_sl^T @ qt')
                atn = attn_pool.tile([P, 8, HALF], mdt, tag="atn")
                for jb in range(8):
                    ps = ps_pool.tile([P, HALF], f32, tag="ps")
                    for c in range(2):
                        nc.tensor.matmul(
                            ps[:],
                            kvin[:, c, P * jb : P * (jb + 1)],
                            qt[:, c, :],
                            start=(c == 0),
                            stop=(c == 1),
                        )
                    nc.scalar.activation(atn[:, jb, :], ps[:], AF.Sigmoid, scale=16.0)
                    if jb == 1 and out_pending is not None:
                        out_proj(h - 1, *out_pending)
                        out_pending = None

                if h + 1 < H:
                    qt_next = q_proj(h + 1, w_next)

                # u_h (f 2x128, i 512) = kvinT_sl^T @ attnT (acc over j)
                u = qkv_pool.tile([P, 2, HALF], mdt, tag="u")
                for t in range(2):
                    ps = ps_pool.tile([P, HALF], f32, tag="ps", name=f"psu{h}{t}")
                    for jb in range(8):
                        nc.tensor.matmul(
                            ps[:],
                            kvt[:, jb, P * t : P * (t + 1)],
                            atn[:, jb, :],
                            start=(jb == 0),
                            stop=(jb == 7),
                        )
                    nc.vector.tensor_copy(u[:, t, :], ps[:])
                out_pending = (w, u)
            out_proj(H - 1, *out_pending)

            # tail: run the two fo-halves' ReLUs on different engines in
            # parallel and overlap the first output DMA with the second.
            out_sb = io_pool.tile([P, 2, HALF], odt, tag="out_sb")
            nc.vector.tensor_relu(out_sb[:, 0, :], out_ps[:, 0, :])
            nc.sync.dma_start(out_d[:, 0], out_sb[:, 0, :])
            nc.scalar.activation(out_sb[:, 1, :], out_ps[:, 1, :], AF.Relu)
            nc.sync.dma_start(out_d[:, 1], out_sb[:, 1, :])

    nc.compile()
    return nc


def _get_nc():
    key = os.environ.get("ATTN_MM_DTYPE", "fp16")
    if key not in _cache:
        _cache[key] = _build()
    return _cache[key]


def _make_in_maps(inputs):
    ndt = (
        np.float16
        if os.environ.get("ATTN_MM_DTYPE", "fp16") == "fp16"
        else np.float32
    )
    q_input = np.asarray(inputs["q_input"], dtype=np.float32)
    kv_input = np.asarray(inputs["kv_input"], dtype=np.float32)

    # Wq/Wk/Wv [f_in, fo*H] (col fi*H + hi) -> [f_in, fo, h]
    WqH = np.asarray(inputs["Wq"], dtype=np.float32).reshape(F, F, H)
    WkH = np.asarray(inputs["Wk"], dtype=np.float32).reshape(F, F, H)
    WvH = np.asarray(inputs["Wv"], dtype=np.float32).reshape(F, F, H)
    # Wz [f*H, fo] (row fi*H + hi) -> [f_in, h, fo]
    WzH = np.asarray(inputs["Wz"], dtype=np.float32).reshape(F, H, F)

    # fold: A_h = Wq_h Wk_h^T, B_h = Wv_h Wz_h  (both [f_in=256, f_out=256])
    # pack each as [P, chunk, f_out]; stack to [H, P, 2(A|B), 2(chunk), F]
    WALL = np.empty((H, P, 2, 2, F), dtype=ndt)
    for h in range(H):
        A = WqH[:, :, h] @ WkH[:, :, h].T
        Bm = WvH[:, :, h] @ WzH[:, h, :]
        WALL[h, :, 0] = A.reshape(2, P, F).transpose(1, 0, 2)
        WALL[h, :, 1] = Bm.reshape(2, P, F).transpose(1, 0, 2)

    in_maps = []
    kvt_c = {}
    kvin_c = {}
    for c in range(NCORES):
        b, half = divmod(c, 2)
        # q_input[b] (256, 1024) -> [p, chunk, i-half]
        qb = q_input[b].reshape(2, P, S)
        qin = np.ascontiguousarray(
            qb[:, :, half * HALF : (half + 1) * HALF].transpose(1, 0, 2), dtype=ndt
        )
        if b not in kvin_c:
            kvin_c[b] = np.ascontiguousarray(
                kv_input[b].reshape(2, P, S).transpose(1, 0, 2), dtype=ndt
            )
            # kvin^T (1024, 256) -> [p, j-block, f]
            kvt_c[b] = np.ascontiguousarray(
                kv_input[b].T.reshape(8, P, F).transpose(1, 0, 2), dtype=ndt
            )
        in_maps.append(
            {"qin": qin, "kvin": kvin_c[b], "kvt": kvt_c[b], "w": WALL}
        )
    return in_maps


def kernel(q_input, kv_input, Wq, Wk, Wv, Wz, **kw):
    from concourse.bass_utils import run_bass_kernel_spmd

    nc = _get_nc()
    in_maps = _make_in_maps(
        {
            "q_input": q_input,
            "kv_input": kv_input,
            "Wq": Wq,
            "Wk": Wk,
            "Wv": Wv,
            "Wz": Wz,
        }
    )

    res = run_bass_kernel_spmd(nc, in_maps, core_ids=list(range(NCORES)))

    out = np.empty((B, F, S), dtype=np.float32)
    for c in range(NCORES):
        b, half = divmod(c, 2)
        # out dram [p, chunk, i] -> out[b, chunk*128+p, half*512+i]
        o = np.asarray(res.results[c]["out"], dtype=np.float32)  # (P, 2, HALF)
        out[b, :, half * HALF : (half + 1) * HALF] = o.transpose(1, 0, 2).reshape(
            F, HALF
        )
    return out


# revision 17
# speedup vs baseline: 1.3080x; 1.0071x over previous
"""Trainium2 Bass kernel for sigmoid-gated multi-head attention.

Reference computation (B=4, F=256, H=8, S=1024):
    qx  = q_input^T          (b, s, f)
    q   = qx @ Wq  -> (b, s, f, h)   [col fi*H + hi]
    k,v = kvx @ Wk / Wv
    attn = sigmoid(sqrt(F) * q.k)    per head
    wv   = attn @ v
    out  = relu(concat_heads(wv) @ Wz)   returned as (b, f, s)

Weight-folding: because attention scores and the output are bilinear in
the projections, the K and V projections can be folded into per-head
256x256 matrices computed on the host for free:
    A_h = Wq_h @ Wk_h^T          qkt_h = qin^T A_h kvin = (A_h^T qin)^T kvin
    B_h = Wv_h @ Wz_h            out  = relu(sum_h B_h^T (kvin @ attnT_h))
This removes the K and V projection matmuls entirely (-28% PE work) and
with them the duplicated K/V compute across the query-half core pair.

Sharding: 8 cores = 4 batches x 2 query-sequence halves. Each core
computes all 8 heads for its (batch, s-half) slice including the final
ReLU, so per-core outputs are disjoint slices of the final output and
no cross-core communication is needed.

Per head (all matmuls fp16, N=512, warm ~216ns):
    qt'_h (f, i)   = A_h^T @ qin                  4 MMs
    attnT_h (j, i) = sigmoid(16 * kvin_sl^T qt')  16 MMs
    u_h   (f, i)   = kvinT_sl^T @ attnT_h         16 MMs (acc over j)
    outT (fo, i)  += B_h^T @ u_h                   4 MMs (acc over h)
Inputs are host-packed partition-major so every DRAM->SBUF transfer is
one large contiguous-per-partition DMA; kvin is supplied in both
feature-major and sequence-major layouts to avoid on-chip transposes.
"""

import os
import sys

sys.path.insert(0, "/opt/trn_rl_repo")

import numpy as np

B, F, H, S = 4, 256, 8, 1024
HALF = S // 2  # query columns per core
NCORES = 8
P = 128  # partitions

_cache = {}


def _build():
    import concourse.mybir as mybir
    import concourse.tile as tile
    from concourse import bacc

    dt = mybir.dt
    f32 = dt.float32
    mm_mode = os.environ.get("ATTN_MM_DTYPE", "fp16")
    mdt = {"fp16": dt.float16, "fp32r": dt.float32r, "fp32": dt.float32}[mm_mode]
    AF = mybir.ActivationFunctionType

    nc = bacc.Bacc(None, target_bir_lowering=False)

    # all partition-major: [P, ...] with per-partition lines contiguous
    qin_d = nc.dram_tensor("qin", [P, 2, HALF], mdt, kind="ExternalInput")
    kvin_d = nc.dram_tensor("kvin", [P, 2, S], mdt, kind="ExternalInput")
    kvt_d = nc.dram_tensor("kvt", [P, 8, F], mdt, kind="ExternalInput")
    # folded weights per head, split so A (needed first, by q_proj) can
    # arrive ahead of B (needed a head later, by out_proj)
    a_d = nc.dram_tensor("wa", [H, P, 2, F], mdt, kind="ExternalInput")
    b_d = nc.dram_tensor("wb", [H, P, 2, F], mdt, kind="ExternalInput")
    odt = dt.float16 if mm_mode == "fp16" else f32
    out_d = nc.dram_tensor("out", [P, 2, HALF], odt, kind="ExternalOutput")

    with tile.TileContext(nc) as tc:
        with (
            tc.tile_pool(name="io", bufs=1) as io_pool,
            tc.tile_pool(name="wts", bufs=3) as w_pool,
            tc.tile_pool(name="qkv", bufs=2) as qkv_pool,
            tc.tile_pool(name="attn", bufs=2) as attn_pool,
            tc.tile_pool(name="ps", bufs=6, space="PSUM") as ps_pool,
            tc.tile_pool(name="ops", bufs=1, space="PSUM") as out_ps_pool,
        ):
            qin = io_pool.tile([P, 2, HALF], mdt, tag="qin")
            kvin = io_pool.tile([P, 2, S], mdt, tag="kvin")
            kvt = io_pool.tile([P, 8, F], mdt, tag="kvt")
            # bulk inputs on the ACT HWDGE ring in need order; all per-head
            # weight tiles ride the otherwise-idle SP ring so triggers are
            # never queued behind sigmoids.
            nc.scalar.dma_start(qin[:], qin_d[:])
            nc.scalar.dma_start(kvin[:], kvin_d[:])
            nc.scalar.dma_start(kvt[:], kvt_d[:])

            # PE pre-warm: dummy matmuls on a zeroed bf16 tile keep the PE
            # busy through its HAM activity window while the first input
            # DMAs are in flight, so the real matmuls start at 2.4 GHz
            # instead of paying the ~3.4us half-clock ramp.
            nwarm = int(os.environ.get("ATTN_NWARM", "10"))
            if nwarm:
                warm = io_pool.tile(
                    [P, HALF], dt.bfloat16 if mm_mode != "fp32" else f32, tag="warm"
                )
                nc.gpsimd.memset(warm[:], 0.0)
                wps = [
                    ps_pool.tile([P, HALF], f32, tag="ps", name=f"wps{i}")
                    for i in range(2)
                ]
                for i in range(nwarm):
                    nc.tensor.matmul(
                        wps[i % 2][:], warm[:, :P], warm[:], start=True, stop=True
                    )

            # persistent accumulator for the output projection: 2 banks
            out_ps = out_ps_pool.tile([P, 2, HALF], f32, tag="out_ps")

            def load_a(h):
                a = w_pool.tile([P, 2, F], mdt, tag="wa", name=f"wa{h}")
                nc.sync.dma_start(a[:], a_d[h])
                return a

            def load_b(h):
                b = w_pool.tile([P, 2, F], mdt, tag="wb", name=f"wb{h}")
                nc.sync.dma_start(b[:], b_d[h])
                return b

            def q_proj(h, a):
                """qt'_h = A_h^T @ qin."""
                qt = qkv_pool.tile([P, 2, HALF], mdt, tag="qt", name=f"qt{h}")
                for t in range(2):
                    ps = ps_pool.tile([P, HALF], f32, tag="ps", name=f"psq{h}{t}")
                    for c in range(2):
                        nc.tensor.matmul(
                            ps[:],
                            a[:, c, P * t : P * (t + 1)],
                            qin[:, c, :],
                            start=(c == 0),
                            stop=(c == 1),
                        )
                    nc.vector.tensor_copy(qt[:, t, :], ps[:])
                return qt

            def out_proj(h, b, u):
                """outT += B_h^T @ u.  c-major so the pair needing only
                u[:,0] runs while u[:,1]'s copy lands."""
                for c in range(2):
                    for t in range(2):
                        nc.tensor.matmul(
                            out_ps[:, t, :],
                            b[:, c, P * t : P * (t + 1)],
                            u[:, c, :],
                            start=(h == 0 and c == 0),
                            stop=(h == H - 1 and c == 1),
                        )

            # software pipeline: weights prefetched a full head ahead; the
            # out-projection of head h-1 is emitted after head h's attention
            # matmuls so its PSUM->SBUF u-copies are long done when the PE
            # reaches it.
            a_next = load_a(0)
            b_cur = load_b(0)
            qt_next = q_proj(0, a_next)
            out_pending = None
            for h in range(H):
                b = b_cur
                qt = qt_next
                if h + 1 < H:
                    a_next = load_a(h + 1)
                    b_cur = load_b(h + 1)
                # attnT_h (j 8x128, i 512) = sigmoid(16 * kvin_sl^T @ qt')
                atn = attn_pool.tile([P, 8, HALF], mdt, tag="atn")
                for jb in range(8):
                    ps = ps_pool.tile([P, HALF], f32, tag="ps")
                    for c in range(2):
                        nc.tensor.matmul(
                            ps[:],
                            kvin[:, c, P * jb : P * (jb + 1)],
                            qt[:, c, :],
                            start=(c == 0),
                            stop=(c == 1),
                        )
                    nc.scalar.activation(atn[:, jb, :], ps[:], AF.Sigmoid, scale=16.0)
                    if jb == 1 and out_pending is not None:
                        out_proj(h - 1, *out_pending)
                        out_pending = None

                if h + 1 < H:
                    qt_next = q_proj(h + 1, a_next)

                # u_h (f 2x128, i 512) = kvinT_sl^T @ attnT (acc over j)
                u = qkv_pool.tile([P, 2, HALF], mdt, tag="u")
                for t in range(2):
                    ps = ps_pool.tile([P, HALF], f32, tag="ps", name=f"psu{h}{t}")
                    for jb in range(8):
                        nc.tensor.matmul(
                            ps[:],
                            kvt[:, jb, P * t : P * (t + 1)],
                            atn[:, jb, :],
                            start=(jb == 0),
                            stop=(jb == 7),
                        )
                    nc.vector.tensor_copy(u[:, t, :], ps[:])
                out_pending = (b, u)
            out_proj(H - 1, *out_pending)

            # tail: run the two fo-halves' ReLUs on different engines in
            # parallel and overlap the first output DMA with the second.
            out_sb = io_pool.tile([P, 2, HALF], odt, tag="out_sb")
            nc.vector.tensor_relu(out_sb[:, 0, :], out_ps[:, 0, :])
            nc.sync.dma_start(out_d[:, 0], out_sb[:, 0, :])
            nc.scalar.activation(out_sb[:, 1, :], out_ps[:, 1, :], AF.Relu)
            nc.sync.dma_start(out_d[:, 1], out_sb[:, 1, :])

    nc.compile()
    return nc


def _get_nc():
    key = os.environ.get("ATTN_MM_DTYPE", "fp16")
    if key not in _cache:
        _cache[key] = _build()
    return _cache[key]


def _make_in_maps(inputs):
    ndt = (
        np.float16
        if os.environ.get("ATTN_MM_DTYPE", "fp16") == "fp16"
        else np.float32
    )
    q_input = np.asarray(inputs["q_input"], dtype=np.float32)
    kv_input = np.asarray(inputs["kv_input"], dtype=np.float32)

    # Wq/Wk/Wv [f_in, fo*H] (col fi*H + hi) -> [f_in, fo, h]
    WqH = np.asarray(inputs["Wq"], dtype=np.float32).reshape(F, F, H)
    WkH = np.asarray(inputs["Wk"], dtype=np.float32).reshape(F, F, H)
    WvH = np.asarray(inputs["Wv"], dtype=np.float32).reshape(F, F, H)
    # Wz [f*H, fo] (row fi*H + hi) -> [f_in, h, fo]
    WzH = np.asarray(inputs["Wz"], dtype=np.float32).reshape(F, H, F)

    # fold: A_h = Wq_h Wk_h^T, B_h = Wv_h Wz_h  (both [f_in=256, f_out=256])
    # pack each as [H, P, chunk, f_out]
    WA = np.empty((H, P, 2, F), dtype=ndt)
    WB = np.empty((H, P, 2, F), dtype=ndt)
    for h in range(H):
        A = WqH[:, :, h] @ WkH[:, :, h].T
        Bm = WvH[:, :, h] @ WzH[:, h, :]
        WA[h] = A.reshape(2, P, F).transpose(1, 0, 2)
        WB[h] = Bm.reshape(2, P, F).transpose(1, 0, 2)

    in_maps = []
    kvt_c = {}
    kvin_c = {}
    for c in range(NCORES):
        b, half = divmod(c, 2)
        # q_input[b] (256, 1024) -> [p, chunk, i-half]
        qb = q_input[b].reshape(2, P, S)
        qin = np.ascontiguousarray(
            qb[:, :, half * HALF : (half + 1) * HALF].transpose(1, 0, 2), dtype=ndt
        )
        if b not in kvin_c:
            kvin_c[b] = np.ascontiguousarray(
                kv_input[b].reshape(2, P, S).transpose(1, 0, 2), dtype=ndt
            )
            # kvin^T (1024, 256) -> [p, j-block, f]
            kvt_c[b] = np.ascontiguousarray(
                kv_input[b].T.reshape(8, P, F).transpose(1, 0, 2), dtype=ndt
            )
        in_maps.append(
            {"qin": qin, "kvin": kvin_c[b], "kvt": kvt_c[b], "wa": WA, "wb": WB}
        )
    return in_maps


def kernel(q_input, kv_input, Wq, Wk, Wv, Wz, **kw):
    from concourse.bass_utils import run_bass_kernel_spmd

    nc = _get_nc()
    in_maps = _make_in_maps(
        {
            "q_input": q_input,
            "kv_input": kv_input,
            "Wq": Wq,
            "Wk": Wk,
            "Wv": Wv,
            "Wz": Wz,
        }
    )

    res = run_bass_kernel_spmd(nc, in_maps, core_ids=list(range(NCORES)))

    out = np.empty((B, F, S), dtype=np.float32)
    for c in range(NCORES):
        b, half = divmod(c, 2)
        # out dram [p, chunk, i] -> out[b, chunk*128+p, half*512+i]
        o = np.asarray(res.results[c]["out"], dtype=np.float32)  # (P, 2, HALF)
        out[b, :, half * HALF : (half + 1) * HALF] = o.transpose(1, 0, 2).reshape(
            F, HALF
        )
    return out


# revision 18
# speedup vs baseline: 1.3343x; 1.0202x over previous
"""Trainium2 Bass kernel for sigmoid-gated multi-head attention.

Reference computation (B=4, F=256, H=8, S=1024):
    qx  = q_input^T          (b, s, f)
    q   = qx @ Wq  -> (b, s, f, h)   [col fi*H + hi]
    k,v = kvx @ Wk / Wv
    attn = sigmoid(sqrt(F) * q.k)    per head
    wv   = attn @ v
    out  = relu(concat_heads(wv) @ Wz)   returned as (b, f, s)

Weight-folding: because attention scores and the output are bilinear in
the projections, the K and V projections can be folded into per-head
256x256 matrices computed on the host for free:
    A_h = Wq_h @ Wk_h^T          qkt_h = qin^T A_h kvin = (A_h^T qin)^T kvin
    B_h = Wv_h @ Wz_h            out  = relu(sum_h B_h^T (kvin @ attnT_h))
This removes the K and V projection matmuls entirely (-28% PE work) and
with them the duplicated K/V compute across the query-half core pair.

Sharding: 8 cores = 4 batches x 2 query-sequence halves. Each core
computes all 8 heads for its (batch, s-half) slice including the final
ReLU, so per-core outputs are disjoint slices of the final output and
no cross-core communication is needed.

Per head (all matmuls fp16, N=512, warm ~216ns):
    qt'_h (f, i)   = A_h^T @ qin                  4 MMs
    attnT_h (j, i) = sigmoid(16 * kvin_sl^T qt')  16 MMs
    u_h   (f, i)   = kvinT_sl^T @ attnT_h         16 MMs (acc over j)
    outT (fo, i)  += B_h^T @ u_h                   4 MMs (acc over h)
Inputs are host-packed partition-major so every DRAM->SBUF transfer is
one large contiguous-per-partition DMA; kvin is supplied in both
feature-major and sequence-major layouts to avoid on-chip transposes.
"""

import os
import sys

sys.path.insert(0, "/opt/trn_rl_repo")

import numpy as np

B, F, H, S = 4, 256, 8, 1024
HALF = S // 2  # query columns per core
NCORES = 8
P = 128  # partitions

_cache = {}


def _build():
    import concourse.mybir as mybir
    import concourse.tile as tile
    from concourse import bacc

    dt = mybir.dt
    f32 = dt.float32
    mm_mode = os.environ.get("ATTN_MM_DTYPE", "fp16")
    mdt = {"fp16": dt.float16, "fp32r": dt.float32r, "fp32": dt.float32}[mm_mode]
    AF = mybir.ActivationFunctionType

    nc = bacc.Bacc(None, target_bir_lowering=False)

    # all partition-major: [P, ...] with per-partition lines contiguous
    qin_d = nc.dram_tensor("qin", [P, 2, HALF], mdt, kind="ExternalInput")
    kvin_d = nc.dram_tensor("kvin", [P, 2, S], mdt, kind="ExternalInput")
    kvt_d = nc.dram_tensor("kvt", [P, 8, F], mdt, kind="ExternalInput")
    # folded weights per head, split so A (needed first, by q_proj) can
    # arrive ahead of B (needed a head later, by out_proj)
    a_d = nc.dram_tensor("wa", [H, P, 2, F], mdt, kind="ExternalInput")
    b_d = nc.dram_tensor("wb", [H, P, 2, F], mdt, kind="ExternalInput")
    odt = dt.float16 if mm_mode == "fp16" else f32
    out_d = nc.dram_tensor("out", [P, 2, HALF], odt, kind="ExternalOutput")

    with tile.TileContext(nc) as tc:
        with (
            tc.tile_pool(name="io", bufs=1) as io_pool,
            tc.tile_pool(name="wts", bufs=3) as w_pool,
            tc.tile_pool(name="qkv", bufs=2) as qkv_pool,
            tc.tile_pool(name="attn", bufs=2) as attn_pool,
            tc.tile_pool(name="ps", bufs=6, space="PSUM") as ps_pool,
            tc.tile_pool(name="ops", bufs=1, space="PSUM") as out_ps_pool,
        ):
            qin = io_pool.tile([P, 2, HALF], mdt, tag="qin")
            kvin = io_pool.tile([P, 2, S], mdt, tag="kvin")
            kvt = io_pool.tile([P, 8, F], mdt, tag="kvt")
            # bulk inputs on the ACT HWDGE ring in need order; all per-head
            # weight tiles ride the otherwise-idle SP ring so triggers are
            # never queued behind sigmoids.
            nc.scalar.dma_start(qin[:], qin_d[:])
            nc.scalar.dma_start(kvin[:], kvin_d[:])
            nc.scalar.dma_start(kvt[:], kvt_d[:])

            # PE pre-warm: dummy matmuls on a zeroed bf16 tile keep the PE
            # busy through its HAM activity window while the first input
            # DMAs are in flight, so the real matmuls start at 2.4 GHz
            # instead of paying the ~3.4us half-clock ramp.
            nwarm = int(os.environ.get("ATTN_NWARM", "10"))
            if nwarm:
                warm = io_pool.tile(
                    [P, HALF], dt.bfloat16 if mm_mode != "fp32" else f32, tag="warm"
                )
                nc.gpsimd.memset(warm[:], 0.0)
                wps = [
                    ps_pool.tile([P, HALF], f32, tag="ps", name=f"wps{i}")
                    for i in range(2)
                ]
                for i in range(nwarm):
                    nc.tensor.matmul(
                        wps[i % 2][:], warm[:, :P], warm[:], start=True, stop=True
                    )

            # persistent accumulator for the output projection: 2 banks
            out_ps = out_ps_pool.tile([P, 2, HALF], f32, tag="out_ps")

            def load_a(h):
                a = w_pool.tile([P, 2, F], mdt, tag="wa", name=f"wa{h}")
                nc.sync.dma_start(a[:], a_d[h])
                return a

            def load_b(h):
                b = w_pool.tile([P, 2, F], mdt, tag="wb", name=f"wb{h}")
                nc.sync.dma_start(b[:], b_d[h])
                return b

            def q_proj(h, a):
                """qt'_h = A_h^T @ qin."""
                qt = qkv_pool.tile([P, 2, HALF], mdt, tag="qt", name=f"qt{h}")
                for t in range(2):
                    ps = ps_pool.tile([P, HALF], f32, tag="ps", name=f"psq{h}{t}")
                    for c in range(2):
                        nc.tensor.matmul(
                            ps[:],
                            a[:, c, P * t : P * (t + 1)],
                            qin[:, c, :],
                            start=(c == 0),
                            stop=(c == 1),
                        )
                    nc.vector.tensor_copy(qt[:, t, :], ps[:])
                return qt

            def out_proj(h, b, u):
                """outT += B_h^T @ u.  c-major so the pair needing only
                u[:,0] runs while u[:,1]'s copy lands."""
                for c in range(2):
                    for t in range(2):
                        nc.tensor.matmul(
                            out_ps[:, t, :],
                            b[:, c, P * t : P * (t + 1)],
                            u[:, c, :],
                            start=(h == 0 and c == 0),
                            stop=(h == H - 1 and c == 1),
                        )

            # software pipeline: weights prefetched a full head ahead; the
            # out-projection of head h-1 is emitted after head h's attention
            # matmuls so its PSUM->SBUF u-copies are long done when the PE
            # reaches it.
            a_next = load_a(0)
            b_cur = load_b(0)
            qt_next = q_proj(0, a_next)
            out_pending = None
            npad = int(os.environ.get("ATTN_NPAD", "120"))
            for h in range(H):
                b = b_cur
                qt = qt_next
                if h + 1 < H:
                    a_next = load_a(h + 1)
                    b_cur = load_b(h + 1)
                if h == H - 1:
                    # pad the PE instruction stream so its final 16KB
                    # IRAM-block boundary is crossed while the array is
                    # still busy: the runtime epilogue block then gets
                    # fetched mid-run instead of stalling the end barrier.
                    # NOPs cost ~20ns of sequencer time each and no array
                    # time (the array keeps draining queued matmuls).
                    for _ in range(npad):
                        nc.tensor.nop(nofuse=True)
                # attnT_h (j 8x128, i 512) = sigmoid(16 * kvin_sl^T @ qt')
                atn = attn_pool.tile([P, 8, HALF], mdt, tag="atn")
                for jb in range(8):
                    ps = ps_pool.tile([P, HALF], f32, tag="ps")
                    for c in range(2):
                        nc.tensor.matmul(
                            ps[:],
                            kvin[:, c, P * jb : P * (jb + 1)],
                            qt[:, c, :],
                            start=(c == 0),
                            stop=(c == 1),
                        )
                    nc.scalar.activation(atn[:, jb, :], ps[:], AF.Sigmoid, scale=16.0)
                    if jb == 1 and out_pending is not None:
                        out_proj(h - 1, *out_pending)
                        out_pending = None

                if h + 1 < H:
                    qt_next = q_proj(h + 1, a_next)

                # u_h (f 2x128, i 512) = kvinT_sl^T @ attnT (acc over j)
                u = qkv_pool.tile([P, 2, HALF], mdt, tag="u")
                for t in range(2):
                    ps = ps_pool.tile([P, HALF], f32, tag="ps", name=f"psu{h}{t}")
                    for jb in range(8):
                        nc.tensor.matmul(
                            ps[:],
                            kvt[:, jb, P * t : P * (t + 1)],
                            atn[:, jb, :],
                            start=(jb == 0),
                            stop=(jb == 7),
                        )
                    nc.vector.tensor_copy(u[:, t, :], ps[:])
                out_pending = (b, u)
            out_proj(H - 1, *out_pending)

            # tail: run the two fo-halves' ReLUs on different engines in
            # parallel and overlap the first output DMA with the second.
            out_sb = io_pool.tile([P, 2, HALF], odt, tag="out_sb")
            nc.vector.tensor_relu(out_sb[:, 0, :], out_ps[:, 0, :])
            nc.sync.dma_start(out_d[:, 0], out_sb[:, 0, :])
            nc.scalar.activation(out_sb[:, 1, :], out_ps[:, 1, :], AF.Relu)
            nc.sync.dma_start(out_d[:, 1], out_sb[:, 1, :])

    nc.compile()
    return nc


def _get_nc():
    key = os.environ.get("ATTN_MM_DTYPE", "fp16")
    if key not in _cache:
        _cache[key] = _build()
    return _cache[key]


def _make_in_maps(inputs):
    ndt = (
        np.float16
        if os.environ.get("ATTN_MM_DTYPE", "fp16") == "fp16"
        else np.float32
    )
    q_input = np.asarray(inputs["q_input"], dtype=np.float32)
    kv_input = np.asarray(inputs["kv_input"], dtype=np.float32)

    # Wq/Wk/Wv [f_in, fo*H] (col fi*H + hi) -> [f_in, fo, h]
    WqH = np.asarray(inputs["Wq"], dtype=np.float32).reshape(F, F, H)
    WkH = np.asarray(inputs["Wk"], dtype=np.float32).reshape(F, F, H)
    WvH = np.asarray(inputs["Wv"], dtype=np.float32).reshape(F, F, H)
    # Wz [f*H, fo] (row fi*H + hi) -> [f_in, h, fo]
    WzH = np.asarray(inputs["Wz"], dtype=np.float32).reshape(F, H, F)

    # fold: A_h = Wq_h Wk_h^T, B_h = Wv_h Wz_h  (both [f_in=256, f_out=256])
    # pack each as [H, P, chunk, f_out]
    WA = np.empty((H, P, 2, F), dtype=ndt)
    WB = np.empty((H, P, 2, F), dtype=ndt)
    for h in range(H):
        A = WqH[:, :, h] @ WkH[:, :, h].T
        Bm = WvH[:, :, h] @ WzH[:, h, :]
        WA[h] = A.reshape(2, P, F).transpose(1, 0, 2)
        WB[h] = Bm.reshape(2, P, F).transpose(1, 0, 2)

    in_maps = []
    kvt_c = {}
    kvin_c = {}
    for c in range(NCORES):
        b, half = divmod(c, 2)
        # q_input[b] (256, 1024) -> [p, chunk, i-half]
        qb = q_input[b].reshape(2, P, S)
        qin = np.ascontiguousarray(
            qb[:, :, half * HALF : (half + 1) * HALF].transpose(1, 0, 2), dtype=ndt
        )
        if b not in kvin_c:
            kvin_c[b] = np.ascontiguousarray(
                kv_input[b].reshape(2, P, S).transpose(1, 0, 2), dtype=ndt
            )
            # kvin^T (1024, 256) -> [p, j-block, f]
            kvt_c[b] = np.ascontiguousarray(
                kv_input[b].T.reshape(8, P, F).transpose(1, 0, 2), dtype=ndt
            )
        in_maps.append(
            {"qin": qin, "kvin": kvin_c[b], "kvt": kvt_c[b], "wa": WA, "wb": WB}
        )
    return in_maps


def kernel(q_input, kv_input, Wq, Wk, Wv, Wz, **kw):
    from concourse.bass_utils import run_bass_kernel_spmd

    nc = _get_nc()
    in_maps = _make_in_maps(
        {
            "q_input": q_input,
            "kv_input": kv_input,
            "Wq": Wq,
            "Wk": Wk,
            "Wv": Wv,
            "Wz": Wz,
        }
    )

    res = run_bass_kernel_spmd(nc, in_maps, core_ids=list(range(NCORES)))

    out = np.empty((B, F, S), dtype=np.float32)
    for c in range(NCORES):
        b, half = divmod(c, 2)
        # out dram [p, chunk, i] -> out[b, chunk*128+p, half*512+i]
        o = np.asarray(res.results[c]["out"], dtype=np.float32)  # (P, 2, HALF)
        out[b, :, half * HALF : (half + 1) * HALF] = o.transpose(1, 0, 2).reshape(
            F, HALF
        )
    return out
